# revision 38
# baseline (speedup 1.0000x reference)
"""CharDecoder LSTM kernel for 8 Trainium2 NeuronCores.

Data-parallel over the flattened (B*W)=8192 batch axis: each of the 8 cores
processes 1024 sequences. Small LSTM/projection weights are replicated; the
hidden/cell state stays resident in SBUF (in transposed [feature, batch]
layout) across all 20 decode steps.

Numerics: all matmuls run in float32r (TF32-like: 8-bit exponent, 11-bit
stored mantissa, 1 cycle/row for free dim >= 256) using an exact Dekker
hi/lo split of both operands:
    w = w1 + w2 (exact, host-side), h = h1 + h2 (exact, 2 DVE ops/tile)
    w @ h ~= w1@h1 + w2@h1 + w1@h2     (dropped w2@h2 term ~ 2^-24)
which is fp32-quality (verified ~1e-6 max rel err end-to-end, 0 argmax
flips) at 3 cycles/row instead of fp32's 4 cycles/row on the PE.

Per-core layout ("layout B", feature-on-partitions):
  h1/h2 splits: [512, 1024] as 4+4 SBUF f32r tiles [128, 1024]
  cT: [512, 1024] as 4 fp32 tiles, updated in place
  gatesT: [2048, 1024] computed as 16 (m) x 2 (n) PSUM tiles [128, 512]
  xT (one-hot next-char): [128 vocab, 1024] f32r, 2 half tiles [128, 512]

argmax -> one-hot per step with zero PE work: GPSIMD partition_all_reduce
(max over the 128 vocab partitions, broadcast to all partitions), then a
DVE is_ge against the logits emitting the exact 0/1 one-hot directly in
[vocab, batch] layout as f32r for the next step's x matmul.
"""

import numpy as np
from contextlib import ExitStack

import concourse.bass as bass
import concourse.bass_isa as bass_isa
import concourse.bacc as bacc
import concourse.mybir as mybir
import concourse.tile as tile
from concourse.bass_utils import run_bass_kernel_spmd

B, W, Q, H, A = 64, 128, 256, 512, 128
C = 20
NCORES = 8
BW = B * W
N_FULL = BW // NCORES  # 1024 batch rows per core

F32 = mybir.dt.float32
F32R = mybir.dt.float32r
AF = mybir.ActivationFunctionType
ALU = mybir.AluOpType

KH = H // 128       # 4 hidden k-tiles
KQ = Q // 128       # 2 input k-tiles
MG = 4 * H // 128   # 16 gate m-tiles

LAST_RESULTS = None  # BassKernelResults of the most recent run (for test.py)


def split12(v):
    """Exact Dekker split of fp32 into two 12-bit-significand (f32r) parts."""
    v = np.ascontiguousarray(np.asarray(v, np.float32))
    u = v.view(np.uint32)
    r = u + (np.uint32(0x7FF) + ((u >> np.uint32(12)) & np.uint32(1)))
    hi = (r & np.uint32(0xFFFFF000)).view(np.float32)
    lo = (v - hi).astype(np.float32)
    return hi, lo


def build_program(c_steps: int = C, n: int = N_FULL) -> bass.Bass:
    assert n % 512 == 0
    nt = n // 512    # PSUM n-tiles per row block

    nc = bacc.Bacc("TRN2", target_bir_lowering=False, debug=False)

    qrT1 = nc.dram_tensor("qrT1", [Q, n], F32R, kind="ExternalInput").ap()
    qrT2 = nc.dram_tensor("qrT2", [Q, n], F32R, kind="ExternalInput").ap()
    w0T1 = nc.dram_tensor("w0T1", [128, KQ * 4 * H], F32R, kind="ExternalInput").ap()
    w0T2 = nc.dram_tensor("w0T2", [128, KQ * 4 * H], F32R, kind="ExternalInput").ap()
    whhT1 = nc.dram_tensor("whhT1", [128, KH * 4 * H], F32R, kind="ExternalInput").ap()
    whhT2 = nc.dram_tensor("whhT2", [128, KH * 4 * H], F32R, kind="ExternalInput").ap()
    wihT1 = nc.dram_tensor("wihT1", [A, 4 * H], F32R, kind="ExternalInput").ap()
    wihT2 = nc.dram_tensor("wihT2", [A, 4 * H], F32R, kind="ExternalInput").ap()
    woutT1 = nc.dram_tensor("woutT1", [128, KH * A], F32R, kind="ExternalInput").ap()
    woutT2 = nc.dram_tensor("woutT2", [128, KH * A], F32R, kind="ExternalInput").ap()
    bias_c = nc.dram_tensor("bias_c", [128, MG], F32, kind="ExternalInput").ap()
    bias0_c = nc.dram_tensor("bias0_c", [128, MG], F32, kind="ExternalInput").ap()
    b_out_c = nc.dram_tensor("b_out_c", [128, 1], F32, kind="ExternalInput").ap()
    out = nc.dram_tensor("out_logits", [c_steps, A, n], F32, kind="ExternalOutput").ap()

    with tile.TileContext(nc) as tc, ExitStack() as ctx:
        wp = ctx.enter_context(tc.tile_pool(name="weights", bufs=1))
        sp = ctx.enter_context(tc.tile_pool(name="state", bufs=2))
        cp = ctx.enter_context(tc.tile_pool(name="cell", bufs=1))
        gp = ctx.enter_context(tc.tile_pool(name="gates", bufs=1))
        tp = ctx.enter_context(tc.tile_pool(name="tmp", bufs=1))
        lp = ctx.enter_context(tc.tile_pool(name="logits", bufs=1))
        xp = ctx.enter_context(tc.tile_pool(name="xhot", bufs=1))
        pg = ctx.enter_context(tc.tile_pool(name="pgate", bufs=6, space="PSUM"))
        pl = ctx.enter_context(tc.tile_pool(name="plog", bufs=2, space="PSUM"))

        # --- DMAs ordered by first use: h0 inputs, then hh weights (k-
        #     interleaved hi/lo to match the k-major accumulation order),
        #     then step-0 logits/argmax operands, then step-1 x operands ---
        h1 = [sp.tile([128, n], F32R, tag=f"h1{j}", name=f"h1{j}") for j in range(KH)]
        h2 = [sp.tile([128, n], F32R, tag=f"h2{j}", name=f"h2{j}") for j in range(KH)]
        ct = [cp.tile([128, n], F32, tag=f"c{j}", name=f"c{j}") for j in range(KH)]
        # step-0 gates come straight from W0 = w_hh @ w_in (host-folded, so
        # h0 is never materialized), as f32r hi/lo 3-product like the rest.
        # The gen-1 h tiles are never written (no h0), so qr hi/lo and the
        # W0-lo halves squat in that dead space; W0-hi borrows the w_ih tag
        # buffers (dead until t=1).
        qr1_t = [h1[k] for k in range(KQ)]         # [128, n] f32r each
        qr2_t = [h2[k] for k in range(KQ)]
        w01_t = [wp.tile([128, 4 * H], F32R, tag=f"wih{k + 1}", name=f"w01{k}")
                 for k in range(KQ)]
        w02_halves = [h1[2], h1[3], h2[2], h2[3]]  # [128, n] each, 2 per k
        bias0_t = wp.tile([128, MG], F32, tag="bias0")
        nc.sync.dma_start(bias0_t[:], bias0_c[:, :])
        # warm the PE's HAM clock gate during the startup DMA wait: tiny
        # self-referential matmuls into an unread PSUM bank, so the 3us
        # cold-clock window expires before the real gate matmuls begin
        wu = pg.tile([128, 512], F32, tag="pg")
        for i in range(280):
            nc.tensor.matmul(wu[0:16, 0:16], lhsT=bias0_t[:, 0:16],
                             rhs=bias0_t[:, 0:16],
                             start=(i == 0), stop=(i == 279))
        for k in range(KQ):
            # operand order matches the first group's product order so the
            # PE starts as early as possible
            nc.sync.dma_start(w01_t[k][:, 0:1024], w0T1[:, k * 4 * H:k * 4 * H + 1024])
            nc.sync.dma_start(qr1_t[k][:, 0:512], qrT1[k * 128:(k + 1) * 128, 0:512])
            nc.sync.dma_start(w02_halves[2 * k][:], w0T2[:, 2 * k * n:(2 * k + 1) * n])
            nc.sync.dma_start(qr2_t[k][:, 0:512], qrT2[k * 128:(k + 1) * 128, 0:512])
            nc.sync.dma_start(w01_t[k][:, 1024:2048],
                              w0T1[:, k * 4 * H + 1024:k * 4 * H + 2048])
            nc.sync.dma_start(w02_halves[2 * k + 1][:],
                              w0T2[:, (2 * k + 1) * n:(2 * k + 2) * n])
            nc.sync.dma_start(qr1_t[k][:, 512:1024], qrT1[k * 128:(k + 1) * 128, 512:1024])
            nc.sync.dma_start(qr2_t[k][:, 512:1024], qrT2[k * 128:(k + 1) * 128, 512:1024])
        whh1_all = wp.tile([128, KH * 4 * H], F32R, tag="whh1")
        nc.sync.dma_start(whh1_all[:], whhT1[:, :])
        whh2_all = wp.tile([128, KH * 4 * H], F32R, tag="whh2")
        nc.sync.dma_start(whh2_all[:], whhT2[:, :])
        whh1_t = [whh1_all[:, k * 4 * H:(k + 1) * 4 * H] for k in range(KH)]
        whh2_t = [whh2_all[:, k * 4 * H:(k + 1) * 4 * H] for k in range(KH)]
        wout1_t = wp.tile([128, KH * A], F32R, tag="wout1")
        nc.sync.dma_start(wout1_t[:], woutT1[:, :])
        wout2_t = wp.tile([128, KH * A], F32R, tag="wout2")
        nc.sync.dma_start(wout2_t[:], woutT2[:, :])
        b_out_t = wp.tile([128, 1], F32, tag="b_out")
        nc.sync.dma_start(b_out_t[:], b_out_c[:, :])
        bias_t = wp.tile([128, MG], F32, tag="bias")
        nc.sync.dma_start(bias_t[:], bias_c[:, :])
        for j in range(KH):
            nc.vector.memset(ct[j][:], 0.0)

        # w_ih reuses the W0 tag buffers; issued after the barrier so its
        # wait-for-W0-reads (t=0 gates) cannot deadlock against the barrier
        wih1_t = wp.tile([128, 4 * H], F32R, tag="wih1")
        nc.sync.dma_start(wih1_t[:], wihT1[:, :])
        wih2_t = wp.tile([128, 4 * H], F32R, tag="wih2")
        nc.sync.dma_start(wih2_t[:], wihT2[:, :])

        xt = None  # one-hot input halves [vocab, batch] f32r; step 0 folds into bias0

        for t in range(c_steps):
            h1n = [sp.tile([128, n], F32R, tag=f"h1{j}", name=f"h1{j}") for j in range(KH)]
            h2n = [sp.tile([128, n], F32R, tag=f"h2{j}", name=f"h2{j}") for j in range(KH)]
            for j in range(KH):
                g_j = [gp.tile([128, n], F32, tag=f"g{q}", name=f"g{q}") for q in range(4)]
                for q in range(4):  # i, f, g, o
                    m = q * KH + j
                    for n0 in range(nt):
                        ps = pg.tile([128, 512], F32, tag="pg")
                        if t == 0:
                            for k in range(KQ):
                                w02 = w02_halves[2 * k + (m // 8)]
                                c0 = (m % 8) * 128
                                nc.tensor.matmul(
                                    ps[:],
                                    lhsT=w01_t[k][:, m * 128:(m + 1) * 128],
                                    rhs=qr1_t[k][:, n0 * 512:(n0 + 1) * 512],
                                    start=(k == 0), stop=False,
                                )
                                nc.tensor.matmul(
                                    ps[:],
                                    lhsT=w02[:, c0:c0 + 128],
                                    rhs=qr1_t[k][:, n0 * 512:(n0 + 1) * 512],
                                    start=False, stop=False,
                                )
                                nc.tensor.matmul(
                                    ps[:],
                                    lhsT=w01_t[k][:, m * 128:(m + 1) * 128],
                                    rhs=qr2_t[k][:, n0 * 512:(n0 + 1) * 512],
                                    start=False, stop=(k == KQ - 1),
                                )
                        else:
                            for k in range(KH):
                                nc.tensor.matmul(
                                    ps[:],
                                    lhsT=whh1_t[k][:, m * 128:(m + 1) * 128],
                                    rhs=h1[k][:, n0 * 512:(n0 + 1) * 512],
                                    start=(k == 0), stop=False,
                                )
                                nc.tensor.matmul(
                                    ps[:],
                                    lhsT=whh2_t[k][:, m * 128:(m + 1) * 128],
                                    rhs=h1[k][:, n0 * 512:(n0 + 1) * 512],
                                    start=False, stop=False,
                                )
                                nc.tensor.matmul(
                                    ps[:],
                                    lhsT=whh1_t[k][:, m * 128:(m + 1) * 128],
                                    rhs=h2[k][:, n0 * 512:(n0 + 1) * 512],
                                    start=False, stop=False,
                                )
                        if t > 0:
                            nc.tensor.matmul(
                                ps[:],
                                lhsT=wih1_t[:, m * 128:(m + 1) * 128],
                                rhs=xt[n0][:],
                                start=False, stop=False,
                            )
                            nc.tensor.matmul(
                                ps[:],
                                lhsT=wih2_t[:, m * 128:(m + 1) * 128],
                                rhs=xt[n0][:],
                                start=False, stop=True,
                            )
                        bias_ap = (bias0_t if t == 0 else bias_t)[:, m:m + 1]
                        func = AF.Tanh if q == 2 else AF.Sigmoid
                        nc.scalar.activation(
                            g_j[q][:, n0 * 512:(n0 + 1) * 512],
                            ps[:], func, bias=bias_ap,
                        )
                # cell/hidden update block j: c = f*c + i*g ; h = o*tanh(c)
                gi, gf, gg, go = (g[:] for g in g_j)
                cs = ct[j][:]
                t1 = tp.tile([128, n], F32, tag="t1")
                nc.vector.tensor_mul(t1[:], gi, gg)
                nc.vector.tensor_mul(cs, gf, cs)
                nc.vector.tensor_add(cs, cs, t1[:])
                t2 = tp.tile([128, n], F32, tag="t2")
                nc.scalar.activation(t2[:], cs, AF.Tanh)
                hf = tp.tile([128, n], F32, tag="t1")
                nc.vector.tensor_mul(hf[:], go, t2[:])
                nc.vector.tensor_copy(h1n[j][:], hf[:])
                nc.vector.tensor_sub(h2n[j][:], hf[:], h1n[j][:].bitcast(F32))

            # logits = w_out @ h + b_out  (in [vocab, batch] layout)
            lg = lp.tile([128, n], F32, tag="logits")
            for n0 in range(nt):
                ps = pl.tile([128, 512], F32, tag="pl")
                for k in range(KH):
                    nc.tensor.matmul(
                        ps[:],
                        lhsT=wout1_t[:, k * A:(k + 1) * A],
                        rhs=h1n[k][:, n0 * 512:(n0 + 1) * 512],
                        start=(k == 0), stop=False,
                    )
                for k in range(KH):
                    nc.tensor.matmul(
                        ps[:],
                        lhsT=wout2_t[:, k * A:(k + 1) * A],
                        rhs=h1n[k][:, n0 * 512:(n0 + 1) * 512],
                        start=False, stop=False,
                    )
                for k in range(KH):
                    nc.tensor.matmul(
                        ps[:],
                        lhsT=wout1_t[:, k * A:(k + 1) * A],
                        rhs=h2n[k][:, n0 * 512:(n0 + 1) * 512],
                        start=False, stop=(k == KH - 1),
                    )
                nc.scalar.activation(
                    lg[:, n0 * 512:(n0 + 1) * 512], ps[:],
                    AF.Identity, bias=b_out_t[:, 0:1],
                )
                nc.sync.dma_start(
                    out[t, :, n0 * 512:(n0 + 1) * 512],
                    lg[:, n0 * 512:(n0 + 1) * 512],
                )

            # next input: one-hot(argmax(logits)) in [vocab, batch] f32r via
            # GPSIMD cross-partition max + DVE is_ge (no PE transposes)
            if t < c_steps - 1:
                xt = [xp.tile([128, 512], F32R, tag=f"x{h}", name=f"x{h}") for h in range(nt)]
                mxb = tp.tile([128, n], F32, tag="t2")
                for n0 in range(nt):
                    nc.gpsimd.partition_all_reduce(
                        mxb[:, n0 * 512:(n0 + 1) * 512],
                        lg[:, n0 * 512:(n0 + 1) * 512],
                        channels=128, reduce_op=bass_isa.ReduceOp.max)
                    nc.vector.tensor_tensor(
                        xt[n0][:],
                        lg[:, n0 * 512:(n0 + 1) * 512],
                        mxb[:, n0 * 512:(n0 + 1) * 512],
                        ALU.is_ge)

            h1, h2 = h1n, h2n

    nc.compile()
    return nc


def make_in_maps(inputs: dict, c_steps: int = C, n: int = N_FULL, ncores: int = NCORES):
    f32 = np.float32
    qr = np.ascontiguousarray(np.asarray(inputs["quantized_repr"], f32)).reshape(BW, Q)
    w_in = np.asarray(inputs["w_in"], f32)
    b_in = np.asarray(inputs["b_in"], f32)
    w_ih = np.asarray(inputs["w_ih"], f32)
    w_hh = np.asarray(inputs["w_hh"], f32)
    b_ih = np.asarray(inputs["b_ih"], f32)
    b_hh = np.asarray(inputs["b_hh"], f32)
    w_out = np.asarray(inputs["w_out"], f32)
    b_out = np.asarray(inputs["b_out"], f32)

    bias = b_ih + b_hh                    # fp32, same as reference
    # step-0 fold: gates0 = w_hh @ (w_in @ qr + b_in) + bias + w_ih[:, 0]
    #            = W0 @ qr + bias0   with W0, bias0 precomputed in fp64
    W0 = (w_hh.astype(np.float64) @ w_in.astype(np.float64)).astype(f32)
    bias0 = (bias.astype(np.float64) + w_ih[:, 0].astype(np.float64)
             + w_hh.astype(np.float64) @ b_in.astype(np.float64)).astype(f32)

    whh1, whh2 = split12(w_hh.T)
    wih1, wih2 = split12(w_ih.T)
    wout1, wout2 = split12(w_out.T)

    def packk(a, ktiles):  # [ktiles*128, cols] -> [128, ktiles*cols]
        cols = a.shape[1]
        out = np.empty((128, ktiles * cols), np.float32)
        for k in range(ktiles):
            out[:, k * cols:(k + 1) * cols] = a[k * 128:(k + 1) * 128, :]
        return np.ascontiguousarray(out)

    w0p1, w0p2 = split12(packk(np.ascontiguousarray(W0.T), KQ))
    shared = {
        "w0T1": w0p1, "w0T2": w0p2,
        "whhT1": packk(whh1, KH), "whhT2": packk(whh2, KH),
        "wihT1": wih1, "wihT2": wih2,
        "woutT1": packk(wout1, KH), "woutT2": packk(wout2, KH),
        "bias_c": np.ascontiguousarray(bias.reshape(MG, 128).T),
        "bias0_c": np.ascontiguousarray(bias0.reshape(MG, 128).T),
        "b_out_c": np.ascontiguousarray(b_out.reshape(128, 1)),
    }
    in_maps = []
    for i in range(ncores):
        m = dict(shared)
        q1, q2 = split12(np.ascontiguousarray(qr[i * n:(i + 1) * n].T))
        m["qrT1"], m["qrT2"] = q1, q2
        in_maps.append(m)
    return in_maps


def kernel(**inputs) -> np.ndarray:
    global LAST_RESULTS
    assert int(inputs["max_char_len"]) == C
    nc = build_program(C, N_FULL)
    in_maps = make_in_maps(inputs, C, N_FULL, NCORES)
    res = run_bass_kernel_spmd(nc, in_maps, core_ids=list(range(NCORES)))
    LAST_RESULTS = res
    # per-core [C, A, N] -> [N, C, A]; concat cores -> [BW, C, A] -> [B, W, C, A]
    parts = [np.transpose(r["out_logits"], (2, 0, 1)) for r in res.results]
    full = np.concatenate(parts, axis=0).reshape(B, W, C, A)
    return np.ascontiguousarray(full)


# revision 41
# speedup vs baseline: 1.0000x; 1.0000x over previous
"""CharDecoder LSTM kernel for 8 Trainium2 NeuronCores.

Data-parallel over the flattened (B*W)=8192 batch axis: each of the 8 cores
processes 1024 sequences. Small LSTM/projection weights are replicated; the
hidden/cell state stays resident in SBUF (in transposed [feature, batch]
layout) across all 20 decode steps.

Numerics: all matmuls run in float32r (TF32-like: 8-bit exponent, 11-bit
stored mantissa, 1 cycle/row for free dim >= 256) using an exact Dekker
hi/lo split of both operands:
    w = w1 + w2 (exact, host-side), h = h1 + h2 (exact, 2 DVE ops/tile)
    w @ h ~= w1@h1 + w2@h1 + w1@h2     (dropped w2@h2 term ~ 2^-24)
which is fp32-quality (verified ~1e-6 max rel err end-to-end, 0 argmax
flips) at 3 cycles/row instead of fp32's 4 cycles/row on the PE.

Per-core layout ("layout B", feature-on-partitions):
  h1/h2 splits: [512, 1024] as 4+4 SBUF f32r tiles [128, 1024]
  cT: [512, 1024] as 4 fp32 tiles, updated in place
  gatesT: [2048, 1024] computed as 16 (m) x 2 (n) PSUM tiles [128, 512]
  xT (one-hot next-char): [128 vocab, 1024] f32r, 2 half tiles [128, 512]

argmax -> one-hot per step with zero PE work: GPSIMD partition_all_reduce
(max over the 128 vocab partitions, broadcast to all partitions), then a
DVE is_ge against the logits emitting the exact 0/1 one-hot directly in
[vocab, batch] layout as f32r for the next step's x matmul.
"""

import numpy as np
from contextlib import ExitStack

import concourse.bass as bass
import concourse.bass_isa as bass_isa
import concourse.bacc as bacc
import concourse.mybir as mybir
import concourse.tile as tile
from concourse.bass_utils import run_bass_kernel_spmd

B, W, Q, H, A = 64, 128, 256, 512, 128
C = 20
NCORES = 8
BW = B * W
N_FULL = BW // NCORES  # 1024 batch rows per core

F32 = mybir.dt.float32
F32R = mybir.dt.float32r
AF = mybir.ActivationFunctionType
ALU = mybir.AluOpType

KH = H // 128       # 4 hidden k-tiles
KQ = Q // 128       # 2 input k-tiles
MG = 4 * H // 128   # 16 gate m-tiles

LAST_RESULTS = None  # BassKernelResults of the most recent run (for test.py)


def split12(v):
    """Exact Dekker split of fp32 into two 12-bit-significand (f32r) parts."""
    v = np.ascontiguousarray(np.asarray(v, np.float32))
    u = v.view(np.uint32)
    r = u + (np.uint32(0x7FF) + ((u >> np.uint32(12)) & np.uint32(1)))
    hi = (r & np.uint32(0xFFFFF000)).view(np.float32)
    lo = (v - hi).astype(np.float32)
    return hi, lo


def build_program(c_steps: int = C, n: int = N_FULL) -> bass.Bass:
    assert n % 512 == 0
    nt = n // 512    # PSUM n-tiles per row block

    nc = bacc.Bacc("TRN2", target_bir_lowering=False, debug=False)

    qrT1 = nc.dram_tensor("qrT1", [Q, n], F32R, kind="ExternalInput").ap()
    qrT2 = nc.dram_tensor("qrT2", [Q, n], F32R, kind="ExternalInput").ap()
    w0T1 = nc.dram_tensor("w0T1", [128, KQ * 4 * H], F32R, kind="ExternalInput").ap()
    w0T2 = nc.dram_tensor("w0T2", [128, KQ * 4 * H], F32R, kind="ExternalInput").ap()
    whhT1 = nc.dram_tensor("whhT1", [128, KH * 4 * H], F32R, kind="ExternalInput").ap()
    whhT2 = nc.dram_tensor("whhT2", [128, KH * 4 * H], F32R, kind="ExternalInput").ap()
    wihT1 = nc.dram_tensor("wihT1", [A, 4 * H], F32R, kind="ExternalInput").ap()
    wihT2 = nc.dram_tensor("wihT2", [A, 4 * H], F32R, kind="ExternalInput").ap()
    woutT1 = nc.dram_tensor("woutT1", [128, KH * A], F32R, kind="ExternalInput").ap()
    woutT2 = nc.dram_tensor("woutT2", [128, KH * A], F32R, kind="ExternalInput").ap()
    bias_c = nc.dram_tensor("bias_c", [128, MG], F32, kind="ExternalInput").ap()
    bias0_c = nc.dram_tensor("bias0_c", [128, MG], F32, kind="ExternalInput").ap()
    b_out_c = nc.dram_tensor("b_out_c", [128, 1], F32, kind="ExternalInput").ap()
    out = nc.dram_tensor("out_logits", [c_steps, A, n], F32, kind="ExternalOutput").ap()

    with tile.TileContext(nc) as tc, ExitStack() as ctx:
        wp = ctx.enter_context(tc.tile_pool(name="weights", bufs=1))
        sp = ctx.enter_context(tc.tile_pool(name="state", bufs=2))
        cp = ctx.enter_context(tc.tile_pool(name="cell", bufs=1))
        gp = ctx.enter_context(tc.tile_pool(name="gates", bufs=1))
        tp = ctx.enter_context(tc.tile_pool(name="tmp", bufs=2))
        lp = ctx.enter_context(tc.tile_pool(name="logits", bufs=1))
        xp = ctx.enter_context(tc.tile_pool(name="xhot", bufs=1))
        pg = ctx.enter_context(tc.tile_pool(name="pgate", bufs=6, space="PSUM"))
        pl = ctx.enter_context(tc.tile_pool(name="plog", bufs=2, space="PSUM"))

        # --- DMAs ordered by first use: h0 inputs, then hh weights (k-
        #     interleaved hi/lo to match the k-major accumulation order),
        #     then step-0 logits/argmax operands, then step-1 x operands ---
        h1 = [sp.tile([128, n], F32R, tag=f"h1{j}", name=f"h1{j}") for j in range(KH)]
        h2 = [sp.tile([128, n], F32R, tag=f"h2{j}", name=f"h2{j}") for j in range(KH)]
        ct = [cp.tile([128, n], F32, tag=f"c{j}", name=f"c{j}") for j in range(KH)]
        # step-0 gates come straight from W0 = w_hh @ w_in (host-folded, so
        # h0 is never materialized), as f32r hi/lo 3-product like the rest.
        # The gen-1 h tiles are never written (no h0), so qr hi/lo and the
        # W0-lo halves squat in that dead space; W0-hi borrows the w_ih tag
        # buffers (dead until t=1).
        qr1_t = [h1[k] for k in range(KQ)]         # [128, n] f32r each
        qr2_t = [h2[k] for k in range(KQ)]
        w01_t = [wp.tile([128, 4 * H], F32R, tag=f"wih{k + 1}", name=f"w01{k}")
                 for k in range(KQ)]
        w02_halves = [h1[2], h1[3], h2[2], h2[3]]  # [128, n] each, 2 per k
        bias0_t = wp.tile([128, MG], F32, tag="bias0")
        nc.sync.dma_start(bias0_t[:], bias0_c[:, :])
        # warm the PE's HAM clock gate during the startup DMA wait: tiny
        # self-referential matmuls into an unread PSUM bank, so the 3us
        # cold-clock window expires before the real gate matmuls begin
        wu = pg.tile([128, 512], F32, tag="pg")
        for i in range(280):
            nc.tensor.matmul(wu[0:16, 0:16], lhsT=bias0_t[:, 0:16],
                             rhs=bias0_t[:, 0:16],
                             start=(i == 0), stop=(i == 279))
        for k in range(KQ):
            # operand order matches the first group's product order so the
            # PE starts as early as possible
            nc.sync.dma_start(w01_t[k][:, 0:1024], w0T1[:, k * 4 * H:k * 4 * H + 1024])
            nc.sync.dma_start(qr1_t[k][:, 0:512], qrT1[k * 128:(k + 1) * 128, 0:512])
            nc.sync.dma_start(w02_halves[2 * k][:], w0T2[:, 2 * k * n:(2 * k + 1) * n])
            nc.sync.dma_start(qr2_t[k][:, 0:512], qrT2[k * 128:(k + 1) * 128, 0:512])
            nc.sync.dma_start(w01_t[k][:, 1024:2048],
                              w0T1[:, k * 4 * H + 1024:k * 4 * H + 2048])
            nc.sync.dma_start(w02_halves[2 * k + 1][:],
                              w0T2[:, (2 * k + 1) * n:(2 * k + 2) * n])
            nc.sync.dma_start(qr1_t[k][:, 512:1024], qrT1[k * 128:(k + 1) * 128, 512:1024])
            nc.sync.dma_start(qr2_t[k][:, 512:1024], qrT2[k * 128:(k + 1) * 128, 512:1024])
        whh1_all = wp.tile([128, KH * 4 * H], F32R, tag="whh1")
        nc.sync.dma_start(whh1_all[:], whhT1[:, :])
        whh2_all = wp.tile([128, KH * 4 * H], F32R, tag="whh2")
        nc.sync.dma_start(whh2_all[:], whhT2[:, :])
        whh1_t = [whh1_all[:, k * 4 * H:(k + 1) * 4 * H] for k in range(KH)]
        whh2_t = [whh2_all[:, k * 4 * H:(k + 1) * 4 * H] for k in range(KH)]
        wout1_t = wp.tile([128, KH * A], F32R, tag="wout1")
        nc.sync.dma_start(wout1_t[:], woutT1[:, :])
        wout2_t = wp.tile([128, KH * A], F32R, tag="wout2")
        nc.sync.dma_start(wout2_t[:], woutT2[:, :])
        b_out_t = wp.tile([128, 1], F32, tag="b_out")
        nc.sync.dma_start(b_out_t[:], b_out_c[:, :])
        bias_t = wp.tile([128, MG], F32, tag="bias")
        nc.sync.dma_start(bias_t[:], bias_c[:, :])
        for j in range(KH):
            nc.vector.memset(ct[j][:], 0.0)

        # w_ih reuses the W0 tag buffers; issued after the barrier so its
        # wait-for-W0-reads (t=0 gates) cannot deadlock against the barrier
        wih1_t = wp.tile([128, 4 * H], F32R, tag="wih1")
        nc.sync.dma_start(wih1_t[:], wihT1[:, :])
        wih2_t = wp.tile([128, 4 * H], F32R, tag="wih2")
        nc.sync.dma_start(wih2_t[:], wihT2[:, :])

        xt = None  # one-hot input halves [vocab, batch] f32r; step 0 folds into bias0

        for t in range(c_steps):
            h1n = [sp.tile([128, n], F32R, tag=f"h1{j}", name=f"h1{j}") for j in range(KH)]
            h2n = [sp.tile([128, n], F32R, tag=f"h2{j}", name=f"h2{j}") for j in range(KH)]
            for j in range(KH):
                g_j = [gp.tile([128, n], F32, tag=f"g{q}", name=f"g{q}") for q in range(4)]
                for q in range(4):  # i, f, g, o
                    m = q * KH + j
                    for n0 in range(nt):
                        ps = pg.tile([128, 512], F32, tag="pg")
                        if t == 0:
                            for k in range(KQ):
                                w02 = w02_halves[2 * k + (m // 8)]
                                c0 = (m % 8) * 128
                                nc.tensor.matmul(
                                    ps[:],
                                    lhsT=w01_t[k][:, m * 128:(m + 1) * 128],
                                    rhs=qr1_t[k][:, n0 * 512:(n0 + 1) * 512],
                                    start=(k == 0), stop=False,
                                )
                                nc.tensor.matmul(
                                    ps[:],
                                    lhsT=w02[:, c0:c0 + 128],
                                    rhs=qr1_t[k][:, n0 * 512:(n0 + 1) * 512],
                                    start=False, stop=False,
                                )
                                nc.tensor.matmul(
                                    ps[:],
                                    lhsT=w01_t[k][:, m * 128:(m + 1) * 128],
                                    rhs=qr2_t[k][:, n0 * 512:(n0 + 1) * 512],
                                    start=False, stop=(k == KQ - 1),
                                )
                        else:
                            for k in range(KH):
                                nc.tensor.matmul(
                                    ps[:],
                                    lhsT=whh1_t[k][:, m * 128:(m + 1) * 128],
                                    rhs=h1[k][:, n0 * 512:(n0 + 1) * 512],
                                    start=(k == 0), stop=False,
                                )
                                nc.tensor.matmul(
                                    ps[:],
                                    lhsT=whh2_t[k][:, m * 128:(m + 1) * 128],
                                    rhs=h1[k][:, n0 * 512:(n0 + 1) * 512],
                                    start=False, stop=False,
                                )
                                nc.tensor.matmul(
                                    ps[:],
                                    lhsT=whh1_t[k][:, m * 128:(m + 1) * 128],
                                    rhs=h2[k][:, n0 * 512:(n0 + 1) * 512],
                                    start=False, stop=False,
                                )
                        if t > 0:
                            nc.tensor.matmul(
                                ps[:],
                                lhsT=wih1_t[:, m * 128:(m + 1) * 128],
                                rhs=xt[n0][:],
                                start=False, stop=False,
                            )
                            nc.tensor.matmul(
                                ps[:],
                                lhsT=wih2_t[:, m * 128:(m + 1) * 128],
                                rhs=xt[n0][:],
                                start=False, stop=True,
                            )
                        bias_ap = (bias0_t if t == 0 else bias_t)[:, m:m + 1]
                        func = AF.Tanh if q == 2 else AF.Sigmoid
                        nc.scalar.activation(
                            g_j[q][:, n0 * 512:(n0 + 1) * 512],
                            ps[:], func, bias=bias_ap,
                        )
                # cell/hidden update block j: c = f*c + i*g ; h = o*tanh(c)
                gi, gf, gg, go = (g[:] for g in g_j)
                cs = ct[j][:]
                t1 = tp.tile([128, n], F32, tag="t1")
                nc.vector.tensor_mul(t1[:], gi, gg)
                nc.vector.tensor_mul(cs, gf, cs)
                nc.vector.tensor_add(cs, cs, t1[:])
                t2 = tp.tile([128, n], F32, tag="t2")
                nc.scalar.activation(t2[:], cs, AF.Tanh)
                hf = tp.tile([128, n], F32, tag="t1")
                nc.vector.tensor_mul(hf[:], go, t2[:])
                nc.vector.tensor_copy(h1n[j][:], hf[:])
                nc.vector.tensor_sub(h2n[j][:], hf[:], h1n[j][:].bitcast(F32))

            # logits = w_out @ h + b_out  (in [vocab, batch] layout)
            lg = lp.tile([128, n], F32, tag="logits")
            for n0 in range(nt):
                ps = pl.tile([128, 512], F32, tag="pl")
                for k in range(KH):
                    nc.tensor.matmul(
                        ps[:],
                        lhsT=wout1_t[:, k * A:(k + 1) * A],
                        rhs=h1n[k][:, n0 * 512:(n0 + 1) * 512],
                        start=(k == 0), stop=False,
                    )
                for k in range(KH):
                    nc.tensor.matmul(
                        ps[:],
                        lhsT=wout2_t[:, k * A:(k + 1) * A],
                        rhs=h1n[k][:, n0 * 512:(n0 + 1) * 512],
                        start=False, stop=False,
                    )
                for k in range(KH):
                    nc.tensor.matmul(
                        ps[:],
                        lhsT=wout1_t[:, k * A:(k + 1) * A],
                        rhs=h2n[k][:, n0 * 512:(n0 + 1) * 512],
                        start=False, stop=(k == KH - 1),
                    )
                nc.scalar.activation(
                    lg[:, n0 * 512:(n0 + 1) * 512], ps[:],
                    AF.Identity, bias=b_out_t[:, 0:1],
                )
                nc.sync.dma_start(
                    out[t, :, n0 * 512:(n0 + 1) * 512],
                    lg[:, n0 * 512:(n0 + 1) * 512],
                )

            # next input: one-hot(argmax(logits)) in [vocab, batch] f32r via
            # GPSIMD cross-partition max + DVE is_ge (no PE transposes)
            if t < c_steps - 1:
                xt = [xp.tile([128, 512], F32R, tag=f"x{h}", name=f"x{h}") for h in range(nt)]
                mxb = tp.tile([128, n], F32, tag="t2")
                for n0 in range(nt):
                    nc.gpsimd.partition_all_reduce(
                        mxb[:, n0 * 512:(n0 + 1) * 512],
                        lg[:, n0 * 512:(n0 + 1) * 512],
                        channels=128, reduce_op=bass_isa.ReduceOp.max)
                    nc.vector.tensor_tensor(
                        xt[n0][:],
                        lg[:, n0 * 512:(n0 + 1) * 512],
                        mxb[:, n0 * 512:(n0 + 1) * 512],
                        ALU.is_ge)

            h1, h2 = h1n, h2n

    nc.compile()
    return nc


def make_in_maps(inputs: dict, c_steps: int = C, n: int = N_FULL, ncores: int = NCORES):
    f32 = np.float32
    qr = np.ascontiguousarray(np.asarray(inputs["quantized_repr"], f32)).reshape(BW, Q)
    w_in = np.asarray(inputs["w_in"], f32)
    b_in = np.asarray(inputs["b_in"], f32)
    w_ih = np.asarray(inputs["w_ih"], f32)
    w_hh = np.asarray(inputs["w_hh"], f32)
    b_ih = np.asarray(inputs["b_ih"], f32)
    b_hh = np.asarray(inputs["b_hh"], f32)
    w_out = np.asarray(inputs["w_out"], f32)
    b_out = np.asarray(inputs["b_out"], f32)

    bias = b_ih + b_hh                    # fp32, same as reference
    # step-0 fold: gates0 = w_hh @ (w_in @ qr + b_in) + bias + w_ih[:, 0]
    #            = W0 @ qr + bias0   with W0, bias0 precomputed in fp64
    W0 = (w_hh.astype(np.float64) @ w_in.astype(np.float64)).astype(f32)
    bias0 = (bias.astype(np.float64) + w_ih[:, 0].astype(np.float64)
             + w_hh.astype(np.float64) @ b_in.astype(np.float64)).astype(f32)

    whh1, whh2 = split12(w_hh.T)
    wih1, wih2 = split12(w_ih.T)
    wout1, wout2 = split12(w_out.T)

    def packk(a, ktiles):  # [ktiles*128, cols] -> [128, ktiles*cols]
        cols = a.shape[1]
        out = np.empty((128, ktiles * cols), np.float32)
        for k in range(ktiles):
            out[:, k * cols:(k + 1) * cols] = a[k * 128:(k + 1) * 128, :]
        return np.ascontiguousarray(out)

    w0p1, w0p2 = split12(packk(np.ascontiguousarray(W0.T), KQ))
    shared = {
        "w0T1": w0p1, "w0T2": w0p2,
        "whhT1": packk(whh1, KH), "whhT2": packk(whh2, KH),
        "wihT1": wih1, "wihT2": wih2,
        "woutT1": packk(wout1, KH), "woutT2": packk(wout2, KH),
        "bias_c": np.ascontiguousarray(bias.reshape(MG, 128).T),
        "bias0_c": np.ascontiguousarray(bias0.reshape(MG, 128).T),
        "b_out_c": np.ascontiguousarray(b_out.reshape(128, 1)),
    }
    in_maps = []
    for i in range(ncores):
        m = dict(shared)
        q1, q2 = split12(np.ascontiguousarray(qr[i * n:(i + 1) * n].T))
        m["qrT1"], m["qrT2"] = q1, q2
        in_maps.append(m)
    return in_maps


def kernel(**inputs) -> np.ndarray:
    global LAST_RESULTS
    assert int(inputs["max_char_len"]) == C
    nc = build_program(C, N_FULL)
    in_maps = make_in_maps(inputs, C, N_FULL, NCORES)
    res = run_bass_kernel_spmd(nc, in_maps, core_ids=list(range(NCORES)))
    LAST_RESULTS = res
    # per-core [C, A, N] -> [N, C, A]; concat cores -> [BW, C, A] -> [B, W, C, A]
    parts = [np.transpose(r["out_logits"], (2, 0, 1)) for r in res.results]
    full = np.concatenate(parts, axis=0).reshape(B, W, C, A)
    return np.ascontiguousarray(full)
